# revision 1
# baseline (speedup 1.0000x reference)
"""GAT layer (LayerNorm -> GATConv(heads=1) -> residual ReLU) on 8 trn2 NeuronCores.

Sharding: destination-node (graph/data) parallel. Each core owns a contiguous
range of N/8 nodes: it computes the node transform for its shard, the shards
are AllGathered so every core holds the full transformed-node table, and each
core then processes the edges whose destination falls in its shard.

Per destination block of 128 nodes, source-node records are fetched with
dma_gather (768 B rows: [xp+bias | 1 | a_src | pad]), per-edge a_dst with a
second dma_gather from a core-local 256 B-row table, attention weights
ee = exp(leakyrelu(a_src + a_dst)) are computed on DVE/ACT, and the
scatter-add is a one-hot matmul: lhsT[e, r] = (iota_r == dstlocal_e) * ee_e
accumulated into PSUM; the table's ones-column yields the softmax denominator
in the same matmuls. Attention/norm parameters are folded on the host into a
single [D,131] matrix + affine row and replicated to every core.
"""

import numpy as np

import concourse.bacc as bacc
import concourse.mybir as mybir
import concourse.tile as tile
from concourse.bass_utils import run_bass_kernel_spmd

F32 = mybir.dt.float32
I16 = mybir.dt.int16
AX = mybir.AxisListType
OP = mybir.AluOpType
AF = mybir.ActivationFunctionType

N = 50000
D = 128
E = 600000
NCORES = 8
SHARD = N // NCORES            # 6250
NBLK = (SHARD + 127) // 128    # 49 dst blocks per core
PAD_SHARD = NBLK * 128         # 6272
LAST_ROWS = SHARD - (NBLK - 1) * 128  # 106
FROW = 192                     # table row f32s (768 B, dma_gather granularity)
AROW = 64                      # a_dst table row f32s (256 B)
GCOL = 130                     # matmul rhs columns: [feat(128) | 1 | a_src]
COL_ONE = 128
COL_ASRC = 129
HALF = 32768                   # int16 index split point for the global table
NEG_SLOPE = 0.2
LN_EPS = 1e-5
GBLK = 2                       # dst blocks per gather group
DEBUG_MAX_GROUPS = None        # limit phase-B groups (bisection aid)
DEBUG_STAGE = 4                # 1=gathers 2=+ee 3=+matmul 4=full (bisection aid)
DEBUG_NO_AG = False            # replace AllGather with local copy (bisection aid)
DEBUG_NO_PHASE_A = False       # stub out phase-A compute (bisection aid)
DEBUG_GATHERS = "both"         # "feat" | "adst" | "both" (bisection aid)


def _build_program(tlo, thi):
    """One SPMD program; per-core behaviour differs only through its inputs.

    tlo/thi: per-block tile counts (of 128 edge slots) for the low/high
    halves of the source table, uniform across cores.
    """
    nc = bacc.Bacc("TRN2", num_devices=NCORES, debug=False)

    CB = sum(tlo) + sum(thi)   # total column-blocks (tiles) per core

    x_shard = nc.dram_tensor("x_shard", [PAD_SHARD, D], F32, kind="ExternalInput")
    wext = nc.dram_tensor("wext", [D, 131], F32, kind="ExternalInput")
    c2b = nc.dram_tensor("c2b", [128, 131], F32, kind="ExternalInput")
    ident = nc.dram_tensor("ident", [128, 128], F32, kind="ExternalInput")
    iota = nc.dram_tensor("iota", [128, 128], F32, kind="ExternalInput")
    feat_idx = nc.dram_tensor("feat_idx", [128, CB * 8], I16, kind="ExternalInput")
    adst_idx = nc.dram_tensor("adst_idx", [128, CB * 8], I16, kind="ExternalInput")
    dloc = nc.dram_tensor("dloc", [128, CB], F32, kind="ExternalInput")
    out_shard = nc.dram_tensor("out_shard", [SHARD, D], F32, kind="ExternalOutput")

    # group structure (static, identical on every core)
    groups = []
    cb0 = 0
    for g0 in range(0, NBLK, GBLK):
        blocks = list(range(g0, min(NBLK, g0 + GBLK)))
        nlo = sum(tlo[b] for b in blocks)
        nhi = sum(thi[b] for b in blocks)
        groups.append((blocks, cb0, nlo, nhi))
        cb0 += nlo + nhi
    assert cb0 == CB
    CBG_MAX = max(nlo + nhi for _, _, nlo, nhi in groups)

    with tile.TileContext(nc) as tc:
        with (
            tc.tile_pool(name="dram", bufs=1, space="DRAM") as dram,
            tc.tile_pool(name="consts", bufs=1) as cpool,
            tc.tile_pool(name="xres", bufs=1) as xpool,
        ):
            xp_shard = dram.tile([SHARD, FROW], F32)
            xp_full = dram.tile([N, FROW], F32, addr_space="Shared")
            adst_loc = dram.tile([PAD_SHARD, AROW], F32)

            ident_sb = cpool.tile([128, 128], F32)
            nc.sync.dma_start(ident_sb[:], ident[:, :])
            iota_sb = cpool.tile([128, 128], F32)
            nc.sync.dma_start(iota_sb[:], iota[:, :])
            wext_sb = cpool.tile([D, 131], F32)
            nc.sync.dma_start(wext_sb[:], wext[:, :])
            c2b_sb = cpool.tile([128, 131], F32)
            nc.sync.dma_start(c2b_sb[:], c2b[:, :])
            eps_sb = cpool.tile([128, 1], F32)
            nc.vector.memset(eps_sb[:], LN_EPS)
            fidx_sb = cpool.tile([128, CB * 8], I16)
            nc.sync.dma_start(fidx_sb[:], feat_idx[:, :])
            aidx_sb = cpool.tile([128, CB * 8], I16)
            nc.sync.dma_start(aidx_sb[:], adst_idx[:, :])
            dl_sb = cpool.tile([128, CB], F32)
            nc.sync.dma_start(dl_sb[:], dloc[:, :])

            x_tiles = []
            for i in range(NBLK):
                xt = xpool.tile([128, D], F32, tag=f"xres{i}")
                nc.sync.dma_start(xt[:], x_shard[i * 128 : (i + 1) * 128, :])
                x_tiles.append(xt)

            # ---------------- Phase A: node transform on own shard ---------
            if DEBUG_NO_PHASE_A:
                nc.sync.dma_start(xp_shard[:, 0:D], x_shard[0:SHARD, :])
                nc.sync.dma_start(
                    adst_loc[0:SHARD, 0:1], x_shard[0:SHARD, 0:1]
                )
            with (
                tc.tile_pool(name="a_small", bufs=8) as spool,
                tc.tile_pool(name="a_sq", bufs=2) as sqpool,
                tc.tile_pool(name="a_xnp", bufs=3) as xnppool,
                tc.tile_pool(name="a_xnpT", bufs=3) as xnptpool,
                tc.tile_pool(name="a_xpe", bufs=3) as xpepool,
                tc.tile_pool(name="a_ps_t", bufs=2, space="PSUM") as psa,
                tc.tile_pool(name="a_ps_m", bufs=2, space="PSUM") as psb,
            ):
                for i in range(NBLK if not DEBUG_NO_PHASE_A else 0):
                    xt = x_tiles[i]
                    rows = 128 if i < NBLK - 1 else LAST_ROWS
                    sumx = spool.tile([128, 1], F32, tag="sumx")
                    nc.vector.tensor_reduce(sumx[:], xt[:], AX.X, OP.add)
                    sqj = sqpool.tile([128, D], F32)
                    ssq = spool.tile([128, 1], F32, tag="ssq")
                    nc.scalar.activation(sqj[:], xt[:], AF.Square, accum_out=ssq[:])
                    mu = spool.tile([128, 1], F32, tag="mu")
                    nc.vector.tensor_scalar(mu[:], sumx[:], 1.0 / D, None, OP.mult)
                    m2 = spool.tile([128, 1], F32, tag="m2")
                    nc.vector.tensor_tensor(m2[:], mu[:], mu[:], OP.mult)
                    var = spool.tile([128, 1], F32, tag="var")
                    nc.vector.tensor_scalar(
                        var[:], ssq[:], 1.0 / D, m2[:, 0:1], OP.mult, OP.subtract
                    )
                    std = spool.tile([128, 1], F32, tag="std")
                    nc.scalar.activation(std[:], var[:], AF.Sqrt, bias=eps_sb[:, 0:1])
                    rstd = spool.tile([128, 1], F32, tag="rstd")
                    nc.vector.reciprocal(rstd[:], std[:])
                    xnp = xnppool.tile([128, D], F32)
                    nc.vector.tensor_scalar(
                        xnp[:], xt[:], mu[:, 0:1], rstd[:, 0:1], OP.subtract, OP.mult
                    )
                    pt = psa.tile([128, 128], F32, space="PSUM")
                    nc.tensor.transpose(pt[:], xnp[:], ident_sb[:])
                    xnpT = xnptpool.tile([128, 128], F32)
                    nc.scalar.copy(xnpT[:], pt[:])
                    pm = psb.tile([128, 131], F32, space="PSUM")
                    nc.tensor.matmul(
                        pm[:], lhsT=xnpT[:], rhs=wext_sb[:], start=True, stop=True
                    )
                    xpe = xpepool.tile([128, 131], F32)
                    nc.vector.tensor_tensor(xpe[:], pm[:], c2b_sb[:], OP.add)
                    nc.sync.dma_start(
                        xp_shard[i * 128 : i * 128 + rows, 0:130], xpe[:rows, 0:130]
                    )
                    nc.sync.dma_start(
                        adst_loc[i * 128 : i * 128 + rows, 0:1], xpe[:rows, 130:131]
                    )

            if DEBUG_NO_AG:
                nc.sync.dma_start(xp_full[0:SHARD, :], xp_shard[:, :])
            else:
                nc.gpsimd.collective_compute(
                    "AllGather",
                    OP.bypass,
                    replica_groups=[list(range(NCORES))],
                    ins=[xp_shard[:, :]],
                    outs=[xp_full[:, :]],
                )

            # ---------------- Phase B: edge aggregation --------------------
            with (
                tc.tile_pool(name="b_g", bufs=2) as gpool,
                tc.tile_pool(name="b_a", bufs=2) as apool,
                tc.tile_pool(name="b_sw", bufs=4) as swpool,
                tc.tile_pool(name="b_e", bufs=3) as epool,
                tc.tile_pool(name="b_ep", bufs=3) as eppool,
                tc.tile_pool(name="b_ps", bufs=4, space="PSUM") as psc,
            ):
                use_groups = groups if DEBUG_MAX_GROUPS is None else groups[:DEBUG_MAX_GROUPS]
                for blocks, cb0, nlo, nhi in use_groups:
                    cbg = nlo + nhi
                    gf = gpool.tile([128, CBG_MAX, FROW], F32, tag="gf")
                    if DEBUG_GATHERS == "adst":
                        nc.vector.memset(gf.rearrange("p a b -> p (a b)")[:], 0.0)
                    if nlo and DEBUG_GATHERS in ("feat", "both"):
                        nc.gpsimd.dma_gather(
                            out_ap=gf[:, 0:nlo, :],
                            in_ap=xp_full[0:HALF, :],
                            idxs_ap=fidx_sb[:, cb0 * 8 : (cb0 + nlo) * 8],
                            num_idxs=nlo * 128,
                            num_idxs_reg=nlo * 128,
                            elem_size=FROW,
                            single_packet=False,
                        )
                    if nhi and DEBUG_GATHERS in ("feat", "both"):
                        nc.gpsimd.dma_gather(
                            out_ap=gf[:, nlo:cbg, :],
                            in_ap=xp_full[HALF:N, :],
                            idxs_ap=fidx_sb[:, (cb0 + nlo) * 8 : (cb0 + cbg) * 8],
                            num_idxs=nhi * 128,
                            num_idxs_reg=nhi * 128,
                            elem_size=FROW,
                            single_packet=False,
                        )
                    ga = apool.tile([128, CBG_MAX, AROW], F32, tag="ga")
                    if DEBUG_GATHERS == "feat":
                        nc.vector.memset(ga.rearrange("p a b -> p (a b)")[:], 1.0)
                    if DEBUG_GATHERS in ("adst", "both"):
                      nc.gpsimd.dma_gather(
                        out_ap=ga[:, 0:cbg, :],
                        in_ap=adst_loc[:, :],
                        idxs_ap=aidx_sb[:, cb0 * 8 : (cb0 + cbg) * 8],
                        num_idxs=cbg * 128,
                        num_idxs_reg=cbg * 128,
                        elem_size=AROW,
                        single_packet=False,
                    )
                    if DEBUG_STAGE < 2:
                        for b in blocks:
                            rows = 128 if b < NBLK - 1 else LAST_ROWS
                            nc.sync.dma_start(
                                out_shard[b * 128 : b * 128 + rows, :],
                                gf[:rows, (b - blocks[0]), 0:D],
                            )
                        continue
                    # ee = exp(leakyrelu(a_src + a_dst)) for the whole group
                    e1 = epool.tile([128, CBG_MAX], F32, tag="e1")
                    nc.vector.tensor_tensor(
                        e1[:, 0:cbg], gf[:, 0:cbg, COL_ASRC], ga[:, 0:cbg, 0], OP.add
                    )
                    e2 = epool.tile([128, CBG_MAX], F32, tag="e2")
                    nc.vector.tensor_scalar(
                        e2[:, 0:cbg], e1[:, 0:cbg], NEG_SLOPE, None, OP.mult
                    )
                    e3 = epool.tile([128, CBG_MAX], F32, tag="e3")
                    nc.vector.tensor_tensor(
                        e3[:, 0:cbg], e2[:, 0:cbg], e1[:, 0:cbg], OP.max
                    )
                    ee = epool.tile([128, CBG_MAX], F32, tag="ee")
                    nc.scalar.activation(ee[:, 0:cbg], e3[:, 0:cbg], AF.Exp)
                    if DEBUG_STAGE < 3:
                        for b in blocks:
                            rows = 128 if b < NBLK - 1 else LAST_ROWS
                            tmp = eppool.tile([128, D], F32, tag="outt")
                            nc.vector.tensor_scalar(
                                tmp[:], iota_sb[:],
                                ee[:, (b - blocks[0]) : (b - blocks[0]) + 1],
                                None, OP.mult,
                            )
                            nc.sync.dma_start(
                                out_shard[b * 128 : b * 128 + rows, :], tmp[:rows, :]
                            )
                        continue

                    # per-block one-hot scatter matmuls
                    lo_off = 0
                    hi_off = nlo
                    for b in blocks:
                        rows = 128 if b < NBLK - 1 else LAST_ROWS
                        cbs = list(range(lo_off, lo_off + tlo[b])) + list(
                            range(hi_off, hi_off + thi[b])
                        )
                        lo_off += tlo[b]
                        hi_off += thi[b]
                        ps = psc.tile([128, GCOL], F32, space="PSUM")
                        for j, cb in enumerate(cbs):
                            sw = swpool.tile([128, 128], F32)
                            nc.vector.tensor_scalar(
                                sw[:],
                                iota_sb[:],
                                dl_sb[:, cb0 + cb : cb0 + cb + 1],
                                ee[:, cb : cb + 1],
                                OP.is_equal,
                                OP.mult,
                            )
                            nc.tensor.matmul(
                                ps[:, :],
                                lhsT=sw[:],
                                rhs=gf[:, cb, 0:GCOL],
                                start=(j == 0),
                                stop=(j == len(cbs) - 1),
                            )
                        if DEBUG_STAGE < 4:
                            tmp = eppool.tile([128, D], F32, tag="outt")
                            nc.vector.tensor_copy(tmp[:], ps[:, 0:D])
                            nc.sync.dma_start(
                                out_shard[b * 128 : b * 128 + rows, :], tmp[:rows, :]
                            )
                            continue
                        recip = epool.tile([128, 1], F32, tag="recip")
                        nc.vector.reciprocal(recip[:], ps[:, COL_ONE : COL_ONE + 1])
                        scaled = eppool.tile([128, D], F32, tag="scaled")
                        nc.scalar.activation(
                            scaled[:], ps[:, 0:D], AF.Copy, scale=recip[:, 0:1]
                        )
                        resid = eppool.tile([128, D], F32, tag="resid")
                        nc.vector.tensor_tensor(
                            resid[:], scaled[:], x_tiles[b][:], OP.add
                        )
                        outt = eppool.tile([128, D], F32, tag="outt")
                        nc.scalar.activation(outt[:], resid[:], AF.Relu)
                        nc.sync.dma_start(
                            out_shard[b * 128 : b * 128 + rows, :], outt[:rows, :]
                        )

    nc.compile()
    return nc


def _wrap_idx(idx):
    """int16 index list -> dma_gather SBUF layout [128, len/16]:
    index i lives at partitions {16g + i%16: g in 0..7}, column i//16."""
    L = len(idx)
    assert L % 16 == 0
    w = idx.reshape(L // 16, 16).T.astype(np.int16)      # [16, L/16]
    return np.tile(w, (8, 1))                            # [128, L/16]


def _host_prep(x, edge_index, ln_gamma, ln_beta, W, att_src, att_dst, bias):
    """Fold parameters and bucket edges by destination block. Numpy only."""
    Wt = W.T.astype(np.float64)
    G = ln_gamma.astype(np.float64)[:, None] * Wt          # [D, D]
    crow = ln_beta.astype(np.float64) @ Wt                 # [D]
    v_src = G @ att_src.astype(np.float64)
    v_dst = G @ att_dst.astype(np.float64)
    c_src = float(crow @ att_src.astype(np.float64))
    c_dst = float(crow @ att_dst.astype(np.float64))

    wext = np.zeros((D, 131), np.float32)
    wext[:, 0:D] = G.astype(np.float32)
    wext[:, COL_ASRC] = v_src.astype(np.float32)
    wext[:, 130] = v_dst.astype(np.float32)
    c2 = np.zeros((131,), np.float32)
    c2[0:D] = (crow + bias.astype(np.float64)).astype(np.float32)
    c2[COL_ONE] = 1.0
    c2[COL_ASRC] = c_src
    c2[130] = c_dst
    c2b = np.broadcast_to(c2, (128, 131)).copy()

    ident = np.eye(128, dtype=np.float32)
    iota = np.broadcast_to(np.arange(128, dtype=np.float32), (128, 128)).copy()

    # edges + self loops, sorted by (core, block, src-half)
    src = np.concatenate([edge_index[0], np.arange(N, dtype=np.int64)]).astype(np.int64)
    dst = np.concatenate([edge_index[1], np.arange(N, dtype=np.int64)]).astype(np.int64)
    core = dst // SHARD
    local = dst - core * SHARD
    blk = local // 128
    half = (src >= HALF).astype(np.int64)
    key = ((core * NBLK + blk) * 2 + half)
    order = np.argsort(key, kind="stable")
    src, dst, key = src[order], dst[order], key[order]
    counts = np.bincount(key, minlength=NCORES * NBLK * 2).reshape(NCORES, NBLK, 2)
    tiles = -(-counts // 128)                              # ceil
    tlo = tuple(int(t) for t in tiles[:, :, 0].max(axis=0))
    thi = tuple(int(t) for t in tiles[:, :, 1].max(axis=0))
    CB = sum(tlo) + sum(thi)

    # per-core slot tables in global column-block (cb) order
    feat_idx = np.zeros((NCORES, CB * 128), np.int16)
    adst_idx = np.zeros((NCORES, CB * 128), np.int16)
    dloc = np.full((NCORES, 128, CB), 128.0, np.float32)

    starts = np.zeros(NCORES * NBLK * 2 + 1, np.int64)
    starts[1:] = np.cumsum(counts.reshape(-1))

    # cb offset of each (block, half) segment, same for every core
    seg_off = {}
    cb0 = 0
    for g0 in range(0, NBLK, GBLK):
        blocks = list(range(g0, min(NBLK, g0 + GBLK)))
        off = cb0
        for b in blocks:
            seg_off[(b, 0)] = off
            off += tlo[b]
        for b in blocks:
            seg_off[(b, 1)] = off
            off += thi[b]
        cb0 = off
    assert cb0 == CB

    for c in range(NCORES):
        for b in range(NBLK):
            for hf in range(2):
                gi = (c * NBLK + b) * 2 + hf
                s, e = starts[gi], starts[gi + 1]
                n = int(e - s)
                if n == 0:
                    continue
                off = seg_off[(b, hf)]
                k = np.arange(n) + off * 128
                fi = (src[s:e] - hf * HALF).astype(np.int16)
                feat_idx[c, k] = fi
                ai = (dst[s:e] - c * SHARD).astype(np.int16)
                adst_idx[c, k] = ai
                p = k % 128
                t = k // 128
                dloc[c, p, t] = (dst[s:e] - (c * SHARD + b * 128)).astype(np.float32)

    in_maps = []
    for c in range(NCORES):
        xs = np.zeros((PAD_SHARD, D), np.float32)
        xs[0:SHARD] = x[c * SHARD : (c + 1) * SHARD]
        in_maps.append(
            {
                "x_shard": xs,
                "wext": wext,
                "c2b": c2b,
                "ident": ident,
                "iota": iota,
                "feat_idx": _wrap_idx(feat_idx[c]),
                "adst_idx": _wrap_idx(adst_idx[c]),
                "dloc": np.ascontiguousarray(dloc[c]),
            }
        )
    return tlo, thi, in_maps


_PROGRAM_CACHE = {}


def kernel(x, edge_index, edge_attr, h, batch, ln_gamma, ln_beta, W, att_src,
           att_dst, bias):
    x = np.asarray(x, dtype=np.float32)
    edge_index = np.asarray(edge_index)
    h = np.asarray(h)
    ln_gamma = np.asarray(ln_gamma, dtype=np.float32)
    ln_beta = np.asarray(ln_beta, dtype=np.float32)
    W = np.asarray(W, dtype=np.float32)
    att_src = np.asarray(att_src, dtype=np.float32)
    att_dst = np.asarray(att_dst, dtype=np.float32)
    bias = np.asarray(bias, dtype=np.float32)

    tlo, thi, in_maps = _host_prep(
        x, edge_index, ln_gamma, ln_beta, W, att_src, att_dst, bias
    )
    key = (tlo, thi)
    if key not in _PROGRAM_CACHE:
        _PROGRAM_CACHE[key] = _build_program(tlo, thi)
    nc = _PROGRAM_CACHE[key]

    res = run_bass_kernel_spmd(nc, in_maps, core_ids=list(range(NCORES)))
    out = np.concatenate([res.results[c]["out_shard"] for c in range(NCORES)], axis=0)
    return out, h



# revision 2
# speedup vs baseline: 1.8755x; 1.8755x over previous
"""GAT layer (LayerNorm -> GATConv(heads=1) -> residual ReLU) on 8 trn2 NeuronCores.

Sharding: destination-node parallel. Each core owns N/8 nodes: phase A computes
the fused node transform for its shard in bf16 ([xp | 1 | a_src] packed into
512 B rows plus a per-block a_dst column kept in SBUF), the shards are
AllGathered into a full bf16 node table, and each core processes the edges
whose destination falls in its shard.

Phase B avoids the per-edge scalar gathers of the naive scheme: only one
dma_gather per edge (512 B row). Per-edge a_dst comes from a 1-column matmul
lhsT=ohT (host-baked transposed one-hot, fp8, streamed sequentially over
HWDGE) against the block's a_dst vector; ee = exp(leakyrelu(a_src + a_dst)) on
DVE/ACT; the scatter-add is a one-hot matmul with bf16 lhsT built on DVE
(iota==dl)*ee and rhs the gathered rows, whose ones-column yields the softmax
denominator in the same matmul.
"""

import numpy as np
import ml_dtypes

import concourse.bacc as bacc
import concourse.mybir as mybir
import concourse.tile as tile
from concourse.bass_utils import run_bass_kernel_spmd

F32 = mybir.dt.float32
BF16 = mybir.dt.bfloat16
FP8 = mybir.dt.float8e4
I16 = mybir.dt.int16
AX = mybir.AxisListType
OP = mybir.AluOpType
AF = mybir.ActivationFunctionType

N = 50000
D = 128
E = 600000
NCORES = 8
SHARD = N // NCORES            # 6250
NBLK = (SHARD + 127) // 128    # 49 dst blocks per core
PAD_SHARD = NBLK * 128         # 6272
LAST_ROWS = SHARD - (NBLK - 1) * 128  # 106
FROW = 256                     # table row, bf16 elems (512 B): [xp|1|a_src|pad]
COL_ONE = 128
COL_ASRC = 129
GROW = 130                     # gathered columns used: [xp(128) | 1 | a_src]
HALF = 32768                   # int16 index split point for the global table
NEG_SLOPE = 0.2
LN_EPS = 1e-5
GBLK = 4                       # dst blocks per gather group

NP_BF16 = ml_dtypes.bfloat16
NP_FP8 = ml_dtypes.float8_e4m3fn


def _build_program(tlo, thi):
    """One SPMD program; per-core behaviour differs only through its inputs.

    tlo/thi: per-block tile counts (of 128 edge slots) for the low/high
    halves of the source table, uniform across cores.
    """
    nc = bacc.Bacc("TRN2", num_devices=NCORES, debug=False)

    CB = sum(tlo) + sum(thi)   # total column-blocks (tiles) per core

    x_shard = nc.dram_tensor("x_shard", [PAD_SHARD, D], F32, kind="ExternalInput")
    wext = nc.dram_tensor("wext", [D, 131], BF16, kind="ExternalInput")
    c2b = nc.dram_tensor("c2b", [128, 131], BF16, kind="ExternalInput")
    ident = nc.dram_tensor("ident", [128, 128], BF16, kind="ExternalInput")
    iota = nc.dram_tensor("iota", [128, 128], BF16, kind="ExternalInput")
    feat_idx = nc.dram_tensor("feat_idx", [128, CB * 8], I16, kind="ExternalInput")
    dloc = nc.dram_tensor("dloc", [128, CB], F32, kind="ExternalInput")
    oht = nc.dram_tensor("oht", [128, CB * 128], FP8, kind="ExternalInput")
    out_shard = nc.dram_tensor("out_shard", [SHARD, D], F32, kind="ExternalOutput")

    # group structure (static, identical on every core)
    groups = []
    cb0 = 0
    for g0 in range(0, NBLK, GBLK):
        blocks = list(range(g0, min(NBLK, g0 + GBLK)))
        nlo = sum(tlo[b] for b in blocks)
        nhi = sum(thi[b] for b in blocks)
        groups.append((blocks, cb0, nlo, nhi))
        cb0 += nlo + nhi
    assert cb0 == CB
    CBG_MAX = max(nlo + nhi for _, _, nlo, nhi in groups)

    with tile.TileContext(nc) as tc:
        with (
            tc.tile_pool(name="dram", bufs=1, space="DRAM") as dram,
            tc.tile_pool(name="consts", bufs=1) as cpool,
            tc.tile_pool(name="xres", bufs=1) as xpool,
        ):
            xp_shard = dram.tile([SHARD, FROW], BF16)
            xp_full = dram.tile([N, FROW], BF16, addr_space="Shared")

            ident_sb = cpool.tile([128, 128], BF16)
            nc.sync.dma_start(ident_sb[:], ident[:, :])
            iota_sb = cpool.tile([128, 128], BF16)
            nc.sync.dma_start(iota_sb[:], iota[:, :])
            wext_sb = cpool.tile([D, 131], BF16)
            nc.sync.dma_start(wext_sb[:], wext[:, :])
            c2b_sb = cpool.tile([128, 131], BF16)
            nc.sync.dma_start(c2b_sb[:], c2b[:, :])
            eps_sb = cpool.tile([128, 1], F32)
            nc.vector.memset(eps_sb[:], LN_EPS)
            fidx_sb = cpool.tile([128, CB * 8], I16)
            nc.sync.dma_start(fidx_sb[:], feat_idx[:, :])
            dl_sb = cpool.tile([128, CB], F32)
            nc.sync.dma_start(dl_sb[:], dloc[:, :])
            adst_sb = cpool.tile([128, NBLK], BF16)

            x_tiles = []
            for i in range(NBLK):
                xt = xpool.tile([128, D], F32, tag=f"xres{i}")
                nc.sync.dma_start(xt[:], x_shard[i * 128 : (i + 1) * 128, :])
                x_tiles.append(xt)

            # ---------------- Phase A: node transform on own shard ---------
            with (
                tc.tile_pool(name="a_small", bufs=8) as spool,
                tc.tile_pool(name="a_sq", bufs=2) as sqpool,
                tc.tile_pool(name="a_xnp", bufs=3) as xnppool,
                tc.tile_pool(name="a_xnpT", bufs=3) as xnptpool,
                tc.tile_pool(name="a_xpe", bufs=3) as xpepool,
                tc.tile_pool(name="a_ps_t", bufs=2, space="PSUM") as psa,
                tc.tile_pool(name="a_ps_m", bufs=2, space="PSUM") as psb,
            ):
                for i in range(NBLK):
                    xt = x_tiles[i]
                    rows = 128 if i < NBLK - 1 else LAST_ROWS
                    sumx = spool.tile([128, 1], F32, tag="sumx")
                    nc.vector.tensor_reduce(sumx[:], xt[:], AX.X, OP.add)
                    sqj = sqpool.tile([128, D], F32)
                    ssq = spool.tile([128, 1], F32, tag="ssq")
                    nc.scalar.activation(sqj[:], xt[:], AF.Square, accum_out=ssq[:])
                    mu = spool.tile([128, 1], F32, tag="mu")
                    nc.vector.tensor_scalar(mu[:], sumx[:], 1.0 / D, None, OP.mult)
                    m2 = spool.tile([128, 1], F32, tag="m2")
                    nc.vector.tensor_tensor(m2[:], mu[:], mu[:], OP.mult)
                    var = spool.tile([128, 1], F32, tag="var")
                    nc.vector.tensor_scalar(
                        var[:], ssq[:], 1.0 / D, m2[:, 0:1], OP.mult, OP.subtract
                    )
                    std = spool.tile([128, 1], F32, tag="std")
                    nc.scalar.activation(std[:], var[:], AF.Sqrt, bias=eps_sb[:, 0:1])
                    rstd = spool.tile([128, 1], F32, tag="rstd")
                    nc.vector.reciprocal(rstd[:], std[:])
                    xnp = xnppool.tile([128, D], BF16)
                    nc.vector.tensor_scalar(
                        xnp[:], xt[:], mu[:, 0:1], rstd[:, 0:1], OP.subtract, OP.mult
                    )
                    pt = psa.tile([128, 128], BF16, space="PSUM")
                    nc.tensor.transpose(pt[:], xnp[:], ident_sb[:])
                    xnpT = xnptpool.tile([128, 128], BF16)
                    nc.scalar.copy(xnpT[:], pt[:])
                    pm = psb.tile([128, 131], F32, space="PSUM")
                    nc.tensor.matmul(
                        pm[:], lhsT=xnpT[:], rhs=wext_sb[:], start=True, stop=True
                    )
                    xpe = xpepool.tile([128, 131], BF16)
                    nc.vector.tensor_tensor(xpe[:], pm[:], c2b_sb[:], OP.add)
                    nc.sync.dma_start(
                        xp_shard[i * 128 : i * 128 + rows, 0:130], xpe[:rows, 0:130]
                    )
                    nc.vector.tensor_copy(adst_sb[:, i : i + 1], xpe[:, 130:131])

            nc.gpsimd.collective_compute(
                "AllGather",
                OP.bypass,
                replica_groups=[list(range(NCORES))],
                ins=[xp_shard[:, :]],
                outs=[xp_full[:, :]],
            )

            # ---------------- Phase B: edge aggregation --------------------
            with (
                tc.tile_pool(name="b_g", bufs=2) as gpool,
                tc.tile_pool(name="b_oht", bufs=2) as opool,
                tc.tile_pool(name="b_sw", bufs=4) as swpool,
                tc.tile_pool(name="b_e", bufs=3) as epool,
                tc.tile_pool(name="b_ep", bufs=3) as eppool,
                tc.tile_pool(name="b_psad", bufs=2, space="PSUM") as psad,
                tc.tile_pool(name="b_ps", bufs=4, space="PSUM") as psc,
            ):
                for blocks, cb0, nlo, nhi in groups:
                    cbg = nlo + nhi
                    gf = gpool.tile([128, CBG_MAX, FROW], BF16, tag="gf")
                    if nlo:
                        nc.gpsimd.dma_gather(
                            out_ap=gf[:, 0:nlo, :],
                            in_ap=xp_full[0:HALF, :],
                            idxs_ap=fidx_sb[:, cb0 * 8 : (cb0 + nlo) * 8],
                            num_idxs=nlo * 128,
                            num_idxs_reg=nlo * 128,
                            elem_size=FROW,
                            single_packet=False,
                        )
                    if nhi:
                        nc.gpsimd.dma_gather(
                            out_ap=gf[:, nlo:cbg, :],
                            in_ap=xp_full[HALF:N, :],
                            idxs_ap=fidx_sb[:, (cb0 + nlo) * 8 : (cb0 + cbg) * 8],
                            num_idxs=nhi * 128,
                            num_idxs_reg=nhi * 128,
                            elem_size=FROW,
                            single_packet=False,
                        )
                    ot = opool.tile([128, CBG_MAX, 128], FP8, tag="ot")
                    nc.sync.dma_start(
                        ot[:, 0:cbg, :].rearrange("p a b -> p (a b)"),
                        oht[:, cb0 * 128 : (cb0 + cbg) * 128],
                    )

                    # per-edge a_dst via 1-col matmuls against the block vector
                    pad = psad.tile([128, CBG_MAX], F32, space="PSUM", tag="pad")
                    lo_off = 0
                    hi_off = nlo
                    tile_of_block = []   # (cb, block) in cb order
                    for b in blocks:
                        for j in range(tlo[b]):
                            tile_of_block.append((lo_off + j, b))
                        lo_off += tlo[b]
                    for b in blocks:
                        for j in range(thi[b]):
                            tile_of_block.append((hi_off + j, b))
                        hi_off += thi[b]
                    for cb, b in tile_of_block:
                        nc.tensor.matmul(
                            pad[:, cb : cb + 1],
                            lhsT=ot[:, cb, :],
                            rhs=adst_sb[:, b : b + 1],
                            start=True,
                            stop=True,
                        )

                    # ee = exp(leakyrelu(a_src + a_dst)) for the whole group
                    e1 = epool.tile([128, CBG_MAX], F32, tag="e1")
                    nc.vector.tensor_tensor(
                        e1[:, 0:cbg], gf[:, 0:cbg, COL_ASRC], pad[:, 0:cbg], OP.add
                    )
                    e2 = epool.tile([128, CBG_MAX], F32, tag="e2")
                    nc.vector.tensor_scalar(
                        e2[:, 0:cbg], e1[:, 0:cbg], NEG_SLOPE, None, OP.mult
                    )
                    e3 = epool.tile([128, CBG_MAX], F32, tag="e3")
                    nc.vector.tensor_tensor(
                        e3[:, 0:cbg], e2[:, 0:cbg], e1[:, 0:cbg], OP.max
                    )
                    ee = epool.tile([128, CBG_MAX], F32, tag="ee")
                    nc.scalar.activation(ee[:, 0:cbg], e3[:, 0:cbg], AF.Exp)

                    # per-block one-hot scatter matmuls (denominator rides as
                    # the ones-column of the gathered rows -> psum col 128)
                    lo_off = 0
                    hi_off = nlo
                    for b in blocks:
                        rows = 128 if b < NBLK - 1 else LAST_ROWS
                        cbs = list(range(lo_off, lo_off + tlo[b])) + list(
                            range(hi_off, hi_off + thi[b])
                        )
                        lo_off += tlo[b]
                        hi_off += thi[b]
                        ps = psc.tile([128, 129], F32, space="PSUM")
                        for j, cb in enumerate(cbs):
                            sw = swpool.tile([128, 128], BF16)
                            nc.vector.tensor_scalar(
                                sw[:],
                                iota_sb[:],
                                dl_sb[:, cb0 + cb : cb0 + cb + 1],
                                ee[:, cb : cb + 1],
                                OP.is_equal,
                                OP.mult,
                            )
                            nc.tensor.matmul(
                                ps[:, :],
                                lhsT=sw[:],
                                rhs=gf[:, cb, 0:129],
                                start=(j == 0),
                                stop=(j == len(cbs) - 1),
                            )
                        recip = epool.tile([128, 1], F32, tag="recip")
                        nc.vector.reciprocal(recip[:], ps[:, COL_ONE : COL_ONE + 1])
                        scaled = eppool.tile([128, D], F32, tag="scaled")
                        nc.scalar.activation(
                            scaled[:], ps[:, 0:D], AF.Copy, scale=recip[:, 0:1]
                        )
                        resid = eppool.tile([128, D], F32, tag="resid")
                        nc.vector.tensor_tensor(
                            resid[:], scaled[:], x_tiles[b][:], OP.add
                        )
                        outt = eppool.tile([128, D], F32, tag="outt")
                        nc.scalar.activation(outt[:], resid[:], AF.Relu)
                        nc.sync.dma_start(
                            out_shard[b * 128 : b * 128 + rows, :], outt[:rows, :]
                        )

    nc.compile()
    return nc


def _wrap_idx(idx):
    """int16 index list -> dma_gather SBUF layout [128, len/16]:
    index i lives at partitions {16g + i%16: g in 0..7}, column i//16."""
    L = len(idx)
    assert L % 16 == 0
    w = idx.reshape(L // 16, 16).T.astype(np.int16)      # [16, L/16]
    return np.tile(w, (8, 1))                            # [128, L/16]


def _host_prep(x, edge_index, ln_gamma, ln_beta, W, att_src, att_dst, bias):
    """Fold parameters and bucket edges by destination block. Numpy only."""
    Wt = W.T.astype(np.float64)
    G = ln_gamma.astype(np.float64)[:, None] * Wt          # [D, D]
    crow = ln_beta.astype(np.float64) @ Wt                 # [D]
    v_src = G @ att_src.astype(np.float64)
    v_dst = G @ att_dst.astype(np.float64)
    c_src = float(crow @ att_src.astype(np.float64))
    c_dst = float(crow @ att_dst.astype(np.float64))

    # xpe columns: [xp+bias (0:128) | 1 (128) | a_src (129) | a_dst (130)]
    wext = np.zeros((D, 131), np.float32)
    wext[:, 0:D] = G.astype(np.float32)
    wext[:, COL_ASRC] = v_src.astype(np.float32)
    wext[:, 130] = v_dst.astype(np.float32)
    c2 = np.zeros((131,), np.float32)
    c2[0:D] = (crow + bias.astype(np.float64)).astype(np.float32)
    c2[COL_ONE] = 1.0
    c2[COL_ASRC] = c_src
    c2[130] = c_dst
    c2b = np.broadcast_to(c2, (128, 131)).copy()

    ident = np.eye(128, dtype=np.float32)
    iota = np.broadcast_to(np.arange(128, dtype=np.float32), (128, 128)).copy()

    # edges + self loops, sorted by (core, block, src-half)
    src = np.concatenate([edge_index[0], np.arange(N, dtype=np.int64)]).astype(np.int64)
    dst = np.concatenate([edge_index[1], np.arange(N, dtype=np.int64)]).astype(np.int64)
    core = dst // SHARD
    local = dst - core * SHARD
    blk = local // 128
    half = (src >= HALF).astype(np.int64)
    key = ((core * NBLK + blk) * 2 + half)
    order = np.argsort(key, kind="stable")
    src, dst, key = src[order], dst[order], key[order]
    counts = np.bincount(key, minlength=NCORES * NBLK * 2).reshape(NCORES, NBLK, 2)
    tiles = -(-counts // 128)                              # ceil
    tlo = tuple(int(t) for t in tiles[:, :, 0].max(axis=0))
    thi = tuple(int(t) for t in tiles[:, :, 1].max(axis=0))
    CB = sum(tlo) + sum(thi)

    # per-core slot tables in global column-block (cb) order
    feat_idx = np.zeros((NCORES, CB * 128), np.int16)
    dloc = np.full((NCORES, 128, CB), 128.0, np.float32)
    # transposed one-hot per tile: oht[c, d, cb, e] = 1 iff dl(slot cb*128+e)==d
    oht = np.zeros((NCORES, 128, CB, 128), NP_FP8)

    starts = np.zeros(NCORES * NBLK * 2 + 1, np.int64)
    starts[1:] = np.cumsum(counts.reshape(-1))

    # cb offset of each (block, half) segment, same for every core
    seg_off = {}
    cb0 = 0
    for g0 in range(0, NBLK, GBLK):
        blocks = list(range(g0, min(NBLK, g0 + GBLK)))
        off = cb0
        for b in blocks:
            seg_off[(b, 0)] = off
            off += tlo[b]
        for b in blocks:
            seg_off[(b, 1)] = off
            off += thi[b]
        cb0 = off
    assert cb0 == CB

    one8 = NP_FP8(1.0)
    for c in range(NCORES):
        for b in range(NBLK):
            for hf in range(2):
                gi = (c * NBLK + b) * 2 + hf
                s, e = starts[gi], starts[gi + 1]
                n = int(e - s)
                if n == 0:
                    continue
                off = seg_off[(b, hf)]
                k = np.arange(n) + off * 128
                fi = (src[s:e] - hf * HALF).astype(np.int16)
                feat_idx[c, k] = fi
                p = k % 128
                t = k // 128
                dl = (dst[s:e] - (c * SHARD + b * 128)).astype(np.int64)
                dloc[c, p, t] = dl.astype(np.float32)
                oht[c, dl, t, p] = one8

    in_maps = []
    for c in range(NCORES):
        xs = np.zeros((PAD_SHARD, D), np.float32)
        xs[0:SHARD] = x[c * SHARD : (c + 1) * SHARD]
        in_maps.append(
            {
                "x_shard": xs,
                "wext": wext.astype(NP_BF16),
                "c2b": c2b.astype(NP_BF16),
                "ident": ident.astype(NP_BF16),
                "iota": iota.astype(NP_BF16),
                "feat_idx": _wrap_idx(feat_idx[c]),
                "dloc": np.ascontiguousarray(dloc[c]),
                "oht": np.ascontiguousarray(oht[c].reshape(128, CB * 128)),
            }
        )
    return tlo, thi, in_maps


_PROGRAM_CACHE = {}


def kernel(x, edge_index, edge_attr, h, batch, ln_gamma, ln_beta, W, att_src,
           att_dst, bias):
    x = np.asarray(x, dtype=np.float32)
    edge_index = np.asarray(edge_index)
    h = np.asarray(h)
    ln_gamma = np.asarray(ln_gamma, dtype=np.float32)
    ln_beta = np.asarray(ln_beta, dtype=np.float32)
    W = np.asarray(W, dtype=np.float32)
    att_src = np.asarray(att_src, dtype=np.float32)
    att_dst = np.asarray(att_dst, dtype=np.float32)
    bias = np.asarray(bias, dtype=np.float32)

    tlo, thi, in_maps = _host_prep(
        x, edge_index, ln_gamma, ln_beta, W, att_src, att_dst, bias
    )
    key = (tlo, thi)
    if key not in _PROGRAM_CACHE:
        _PROGRAM_CACHE[key] = _build_program(tlo, thi)
    nc = _PROGRAM_CACHE[key]

    res = run_bass_kernel_spmd(nc, in_maps, core_ids=list(range(NCORES)))
    out = np.concatenate([res.results[c]["out_shard"] for c in range(NCORES)], axis=0)
    return out, h


# revision 6
# speedup vs baseline: 2.6162x; 1.3949x over previous
"""GAT layer (LayerNorm -> GATConv(heads=1) -> residual ReLU) on 8 trn2 NeuronCores.

Sharding: destination-node parallel. Each core owns N/8 nodes: phase A computes
the fused node transform for its shard in bf16 ([xp | 1 | a_src] packed into
512 B table rows; a_src/a_dst block columns kept in SBUF), the shards are
AllGathered into a full bf16 node table (split into two collectives aligned
with the int16 lo/hi index halves so low-half gathers start earlier), and each
core processes the edges whose destination falls in its shard.

Phase B does exactly one dma_gather descriptor per non-self-loop edge (512 B
row), rotating gather calls over 4 SWDGE queues. Self-loop contributions come
straight from the SBUF-resident phase-A tiles (no gather). Per-edge a_dst is a
1-column matmul of a host-baked transposed one-hot (fp8, streamed over HWDGE)
against the block's a_dst column; ee = exp(leakyrelu(a_src + a_dst)); the
scatter-add is a one-hot matmul with bf16 lhsT (iota==dl)*ee whose rhs
ones-column yields the softmax denominator in the same matmul.
"""

import numpy as np
import ml_dtypes

import concourse.bacc as bacc
import concourse.mybir as mybir
import concourse.tile as tile
from concourse.bass_utils import run_bass_kernel_spmd

F32 = mybir.dt.float32
BF16 = mybir.dt.bfloat16
FP8 = mybir.dt.float8e4
I16 = mybir.dt.int16
AX = mybir.AxisListType
OP = mybir.AluOpType
AF = mybir.ActivationFunctionType

N = 50000
D = 128
E = 600000
NCORES = 8
SHARD = N // NCORES            # 6250
NBLK = (SHARD + 127) // 128    # 49 dst blocks per core
PAD_SHARD = NBLK * 128         # 6272
LAST_ROWS = SHARD - (NBLK - 1) * 128  # 106
FROW = 256                     # table row, bf16 elems (512 B): [xp|1|a_src|pad]
COL_ONE = 128
COL_ASRC = 129
HALF = 32768                   # int16 index split point for the global table
LSPLIT = HALF // NCORES        # 4096 low-half rows per core
HSPLIT = SHARD - LSPLIT        # 2154 high-half rows per core
NEG_SLOPE = 0.2
LN_EPS = 1e-5
GBLK = 4                       # dst blocks per gather group
NQ = 4                         # SWDGE queues rotated across gather calls

NP_BF16 = ml_dtypes.bfloat16
NP_FP8 = ml_dtypes.float8_e4m3fn


def _build_program(tlo, thi):
    """One SPMD program; per-core behaviour differs only through its inputs.

    tlo/thi: per-block tile counts (of 128 edge slots, self loops excluded)
    for the low/high halves of the source table, uniform across cores.
    """
    nc = bacc.Bacc("TRN2", num_devices=NCORES, debug=False, num_swdge_queues=NQ)

    CB = sum(tlo) + sum(thi)   # total column-blocks (tiles) per core

    x_shard = nc.dram_tensor("x_shard", [PAD_SHARD, D], F32, kind="ExternalInput")
    wext = nc.dram_tensor("wext", [D, 131], BF16, kind="ExternalInput")
    c2row = nc.dram_tensor("c2row", [1, 131], BF16, kind="ExternalInput")
    ones1 = nc.dram_tensor("ones1", [1, 128], BF16, kind="ExternalInput")
    ident = nc.dram_tensor("ident", [128, 128], BF16, kind="ExternalInput")
    iota = nc.dram_tensor("iota", [128, 128], BF16, kind="ExternalInput")
    iotap = nc.dram_tensor("iotap", [128, 1], F32, kind="ExternalInput")
    feat_idx = nc.dram_tensor("feat_idx", [128, CB * 8], I16, kind="ExternalInput")
    dloc = nc.dram_tensor("dloc", [128, CB], F32, kind="ExternalInput")
    oht = nc.dram_tensor("oht", [128, CB * 128], FP8, kind="ExternalInput")
    out_shard = nc.dram_tensor("out_shard", [SHARD, D], F32, kind="ExternalOutput")

    # group structure (static, identical on every core)
    groups = []
    cb0 = 0
    for g0 in range(0, NBLK, GBLK):
        blocks = list(range(g0, min(NBLK, g0 + GBLK)))
        nlo = sum(tlo[b] for b in blocks)
        nhi = sum(thi[b] for b in blocks)
        groups.append((blocks, cb0, nlo, nhi))
        cb0 += nlo + nhi
    assert cb0 == CB
    CBG_MAX = max(nlo + nhi for _, _, nlo, nhi in groups)

    with tile.TileContext(nc) as tc:
        with (
            tc.tile_pool(name="dram", bufs=1, space="DRAM") as dram,
            tc.tile_pool(name="consts", bufs=1) as cpool,
            tc.tile_pool(name="xres", bufs=1) as xpool,
            tc.tile_pool(name="xpe", bufs=1) as xpepool,
        ):
            xp_shard = dram.tile([SHARD, FROW], BF16)
            xp_lo = dram.tile([HALF, FROW], BF16, addr_space="Shared")
            xp_hi = dram.tile([N - HALF, FROW], BF16, addr_space="Shared")

            ident_sb = cpool.tile([128, 128], BF16)
            nc.sync.dma_start(ident_sb[:], ident[:, :])
            iota_sb = cpool.tile([128, 128], BF16)
            nc.sync.dma_start(iota_sb[:], iota[:, :])
            iotap_sb = cpool.tile([128, 1], F32)
            nc.sync.dma_start(iotap_sb[:], iotap[:, :])
            wext_sb = cpool.tile([D, 131], BF16)
            nc.sync.dma_start(wext_sb[:], wext[:, :])
            c2row_sb = cpool.tile([1, 131], BF16)
            nc.sync.dma_start(c2row_sb[:], c2row[:, :])
            ones1_sb = cpool.tile([1, 128], BF16)
            nc.sync.dma_start(ones1_sb[:], ones1[:, :])
            eps_sb = cpool.tile([128, 1], F32)
            nc.vector.memset(eps_sb[:], LN_EPS)
            fidx_sb = cpool.tile([128, CB * 8], I16)
            nc.sync.dma_start(fidx_sb[:], feat_idx[:, :])
            dl_sb = cpool.tile([128, CB], F32)
            nc.sync.dma_start(dl_sb[:], dloc[:, :])
            asrc_sb = cpool.tile([128, NBLK], F32)
            adst_sb = cpool.tile([128, NBLK], BF16)

            x_tiles = []
            for i in range(NBLK):
                xt = xpool.tile([128, D], F32, tag=f"xres{i}")
                nc.sync.dma_start(xt[:], x_shard[i * 128 : (i + 1) * 128, :])
                x_tiles.append(xt)

            # ---------------- Phase A: node transform on own shard ---------
            xpe_tiles = []
            with (
                tc.tile_pool(name="a_small", bufs=8) as spool,
                tc.tile_pool(name="a_sq", bufs=4) as sqpool,
                tc.tile_pool(name="a_xnp", bufs=4) as xnppool,
                tc.tile_pool(name="a_xnpT", bufs=4) as xnptpool,
                tc.tile_pool(name="a_ps_t", bufs=3, space="PSUM") as psa,
                tc.tile_pool(name="a_ps_m", bufs=3, space="PSUM") as psb,
            ):
                for i in range(NBLK):
                    xt = x_tiles[i]
                    rows = 128 if i < NBLK - 1 else LAST_ROWS
                    sumx = spool.tile([128, 1], F32, tag="sumx")
                    nc.vector.tensor_reduce(sumx[:], xt[:], AX.X, OP.add)
                    sqj = sqpool.tile([128, D], F32)
                    ssq = spool.tile([128, 1], F32, tag="ssq")
                    nc.scalar.activation(sqj[:], xt[:], AF.Square, accum_out=ssq[:])
                    mu = spool.tile([128, 1], F32, tag="mu")
                    nc.vector.tensor_scalar(mu[:], sumx[:], 1.0 / D, None, OP.mult)
                    m2 = spool.tile([128, 1], F32, tag="m2")
                    nc.vector.tensor_tensor(m2[:], mu[:], mu[:], OP.mult)
                    var = spool.tile([128, 1], F32, tag="var")
                    nc.vector.tensor_scalar(
                        var[:], ssq[:], 1.0 / D, m2[:, 0:1], OP.mult, OP.subtract
                    )
                    std = spool.tile([128, 1], F32, tag="std")
                    nc.scalar.activation(std[:], var[:], AF.Sqrt, bias=eps_sb[:, 0:1])
                    rstd = spool.tile([128, 1], F32, tag="rstd")
                    nc.vector.reciprocal(rstd[:], std[:])
                    xnp = xnppool.tile([128, D], BF16)
                    nc.vector.tensor_scalar(
                        xnp[:], xt[:], mu[:, 0:1], rstd[:, 0:1], OP.subtract, OP.mult
                    )
                    pt = psa.tile([128, 128], BF16, space="PSUM")
                    nc.tensor.transpose(pt[:], xnp[:], ident_sb[:])
                    xnpT = xnptpool.tile([128, 128], BF16)
                    nc.scalar.copy(xnpT[:], pt[:])
                    pm = psb.tile([128, 131], F32, space="PSUM")
                    nc.tensor.matmul(
                        pm[:], lhsT=ones1_sb[:, :], rhs=c2row_sb[:, :],
                        start=True, stop=False,
                    )
                    nc.tensor.matmul(
                        pm[:], lhsT=xnpT[:], rhs=wext_sb[:], start=False, stop=True
                    )
                    xpe = xpepool.tile([128, 131], BF16, tag=f"xpe{i}")
                    nc.scalar.copy(xpe[:], pm[:])
                    xpe_tiles.append(xpe)
                    nc.sync.dma_start(
                        xp_shard[i * 128 : i * 128 + rows, 0:130], xpe[:rows, 0:130]
                    )
                    nc.vector.tensor_copy(asrc_sb[:, i : i + 1], pm[:, 129:130])
                    nc.vector.tensor_copy(adst_sb[:, i : i + 1], pm[:, 130:131])

            # split AllGather aligned with the lo/hi table halves
            nc.gpsimd.collective_compute(
                "AllGather",
                OP.bypass,
                replica_groups=[list(range(NCORES))],
                ins=[xp_shard[0:LSPLIT, :]],
                outs=[xp_lo[:, :]],
            )
            nc.gpsimd.collective_compute(
                "AllGather",
                OP.bypass,
                replica_groups=[list(range(NCORES))],
                ins=[xp_shard[LSPLIT:SHARD, :]],
                outs=[xp_hi[:, :]],
            )

            # ---------------- Phase B: edge aggregation --------------------
            with (
                tc.tile_pool(name="b_g", bufs=3) as gpool,
                tc.tile_pool(name="b_oht", bufs=3) as opool,
                tc.tile_pool(name="b_sw", bufs=4) as swpool,
                tc.tile_pool(name="b_e", bufs=3) as epool,
                tc.tile_pool(name="b_ep", bufs=3) as eppool,
                tc.tile_pool(name="b_psad", bufs=2, space="PSUM") as psad,
                tc.tile_pool(name="b_ps", bufs=4, space="PSUM") as psc,
            ):
                # self-loop attention logits for all blocks at once
                e1s = epool.tile([128, NBLK], F32, tag="e1s")
                nc.vector.tensor_tensor(e1s[:], asrc_sb[:], adst_sb[:], OP.add)
                e3s = epool.tile([128, NBLK], F32, tag="e3s")
                nc.vector.scalar_tensor_tensor(
                    e3s[:], e1s[:], NEG_SLOPE, e1s[:], OP.mult, OP.max
                )
                ees = epool.tile([128, NBLK], F32, tag="ees")
                nc.scalar.activation(ees[:], e3s[:], AF.Exp)

                qctr = 0
                for blocks, cb0, nlo, nhi in groups:
                    cbg = nlo + nhi
                    gf = gpool.tile([128, CBG_MAX, FROW], BF16, tag="gf")
                    if nlo:
                        nc.gpsimd.dma_gather(
                            out_ap=gf[:, 0:nlo, :],
                            in_ap=xp_lo[:, :],
                            idxs_ap=fidx_sb[:, cb0 * 8 : (cb0 + nlo) * 8],
                            num_idxs=nlo * 128,
                            num_idxs_reg=nlo * 128,
                            elem_size=FROW,
                            single_packet=False,
                            queue_num=qctr % NQ,
                        )
                        qctr += 1
                    if nhi:
                        nc.gpsimd.dma_gather(
                            out_ap=gf[:, nlo:cbg, :],
                            in_ap=xp_hi[:, :],
                            idxs_ap=fidx_sb[:, (cb0 + nlo) * 8 : (cb0 + cbg) * 8],
                            num_idxs=nhi * 128,
                            num_idxs_reg=nhi * 128,
                            elem_size=FROW,
                            single_packet=False,
                            queue_num=qctr % NQ,
                        )
                        qctr += 1
                    ot = opool.tile([128, CBG_MAX, 128], FP8, tag="ot")
                    nc.sync.dma_start(
                        ot[:, 0:cbg, :].rearrange("p a b -> p (a b)"),
                        oht[:, cb0 * 128 : (cb0 + cbg) * 128],
                    )

                    # per-edge a_dst via 1-col matmuls against the block vector
                    pad = psad.tile([128, CBG_MAX], F32, space="PSUM", tag="pad")
                    lo_off = 0
                    hi_off = nlo
                    tile_of_block = []   # (cb, block) in cb order
                    for b in blocks:
                        for j in range(tlo[b]):
                            tile_of_block.append((lo_off + j, b))
                        lo_off += tlo[b]
                    for b in blocks:
                        for j in range(thi[b]):
                            tile_of_block.append((hi_off + j, b))
                        hi_off += thi[b]
                    for cb, b in tile_of_block:
                        nc.tensor.matmul(
                            pad[:, cb : cb + 1],
                            lhsT=ot[:, cb, :],
                            rhs=adst_sb[:, b : b + 1],
                            start=True,
                            stop=True,
                        )

                    # ee = exp(leakyrelu(a_src + a_dst)) for the whole group
                    e1 = epool.tile([128, CBG_MAX], F32, tag="e1")
                    nc.vector.tensor_tensor(
                        e1[:, 0:cbg], gf[:, 0:cbg, COL_ASRC], pad[:, 0:cbg], OP.add
                    )
                    e3 = epool.tile([128, CBG_MAX], F32, tag="e3")
                    nc.vector.scalar_tensor_tensor(
                        e3[:, 0:cbg], e1[:, 0:cbg], NEG_SLOPE, e1[:, 0:cbg],
                        OP.mult, OP.max,
                    )
                    ee = epool.tile([128, CBG_MAX], F32, tag="ee")
                    nc.scalar.activation(ee[:, 0:cbg], e3[:, 0:cbg], AF.Exp)

                    # per-block one-hot scatter matmuls; chain starts with the
                    # SBUF-resident self-loop tile (denominator rides as the
                    # ones-column of the rhs -> psum col 128)
                    lo_off = 0
                    hi_off = nlo
                    for b in blocks:
                        rows = 128 if b < NBLK - 1 else LAST_ROWS
                        cbs = list(range(lo_off, lo_off + tlo[b])) + list(
                            range(hi_off, hi_off + thi[b])
                        )
                        lo_off += tlo[b]
                        hi_off += thi[b]
                        ps = psc.tile([128, 129], F32, space="PSUM")
                        swd = swpool.tile([128, 128], BF16)
                        nc.vector.tensor_scalar(
                            swd[:],
                            iota_sb[:],
                            iotap_sb[:, 0:1],
                            ees[:, b : b + 1],
                            OP.is_equal,
                            OP.mult,
                        )
                        nc.tensor.matmul(
                            ps[:, :],
                            lhsT=swd[:],
                            rhs=xpe_tiles[b][:, 0:129],
                            start=True,
                            stop=(len(cbs) == 0),
                        )
                        for j, cb in enumerate(cbs):
                            sw = swpool.tile([128, 128], BF16)
                            nc.vector.tensor_scalar(
                                sw[:],
                                iota_sb[:],
                                dl_sb[:, cb0 + cb : cb0 + cb + 1],
                                ee[:, cb : cb + 1],
                                OP.is_equal,
                                OP.mult,
                            )
                            nc.tensor.matmul(
                                ps[:, :],
                                lhsT=sw[:],
                                rhs=gf[:, cb, 0:129],
                                start=False,
                                stop=(j == len(cbs) - 1),
                            )
                        recip = epool.tile([128, 1], F32, tag="recip")
                        nc.vector.reciprocal(recip[:], ps[:, COL_ONE : COL_ONE + 1])
                        scaled = eppool.tile([128, D], F32, tag="scaled")
                        nc.scalar.activation(
                            scaled[:], ps[:, 0:D], AF.Copy, scale=recip[:, 0:1]
                        )
                        resid = eppool.tile([128, D], F32, tag="resid")
                        nc.vector.tensor_tensor(
                            resid[:], scaled[:], x_tiles[b][:], OP.add
                        )
                        outt = eppool.tile([128, D], F32, tag="outt")
                        nc.scalar.activation(outt[:], resid[:], AF.Relu)
                        nc.sync.dma_start(
                            out_shard[b * 128 : b * 128 + rows, :], outt[:rows, :]
                        )

    nc.compile()
    return nc


def _wrap_idx(idx):
    """int16 index list -> dma_gather SBUF layout [128, len/16]:
    index i lives at partitions {16g + i%16: g in 0..7}, column i//16."""
    L = len(idx)
    assert L % 16 == 0
    w = idx.reshape(L // 16, 16).T.astype(np.int16)      # [16, L/16]
    return np.tile(w, (8, 1))                            # [128, L/16]


def _table_row(node):
    """Node id -> row in the AllGathered table (lo/hi-split layout)."""
    c = node // SHARD
    i = node - c * SHARD
    return np.where(i < LSPLIT, c * LSPLIT + i, HALF + c * HSPLIT + (i - LSPLIT))


def _host_prep(x, edge_index, ln_gamma, ln_beta, W, att_src, att_dst, bias):
    """Fold parameters and bucket edges by destination block. Numpy only."""
    Wt = W.T.astype(np.float64)
    G = ln_gamma.astype(np.float64)[:, None] * Wt          # [D, D]
    crow = ln_beta.astype(np.float64) @ Wt                 # [D]
    v_src = G @ att_src.astype(np.float64)
    v_dst = G @ att_dst.astype(np.float64)
    c_src = float(crow @ att_src.astype(np.float64))
    c_dst = float(crow @ att_dst.astype(np.float64))

    # xpe columns: [xp+bias (0:128) | 1 (128) | a_src (129) | a_dst (130)]
    wext = np.zeros((D, 131), np.float32)
    wext[:, 0:D] = G.astype(np.float32)
    wext[:, COL_ASRC] = v_src.astype(np.float32)
    wext[:, 130] = v_dst.astype(np.float32)
    c2 = np.zeros((1, 131), np.float32)
    c2[0, 0:D] = (crow + bias.astype(np.float64)).astype(np.float32)
    c2[0, COL_ONE] = 1.0
    c2[0, COL_ASRC] = c_src
    c2[0, 130] = c_dst

    ident = np.eye(128, dtype=np.float32)
    iota = np.broadcast_to(np.arange(128, dtype=np.float32), (128, 128)).copy()
    iotap = np.arange(128, dtype=np.float32).reshape(128, 1)
    ones1 = np.ones((1, 128), np.float32)

    # edges (self loops are handled from SBUF, not gathered), sorted by
    # (core, block, src-half, src-row) - src order improves HBM locality
    src = edge_index[0].astype(np.int64)
    dst = edge_index[1].astype(np.int64)
    trow = _table_row(src)
    core = dst // SHARD
    local = dst - core * SHARD
    blk = local // 128
    half = (trow >= HALF).astype(np.int64)
    key = ((core * NBLK + blk) * 2 + half)
    order = np.lexsort((trow, key))
    src, dst, key, trow = src[order], dst[order], key[order], trow[order]
    counts = np.bincount(key, minlength=NCORES * NBLK * 2).reshape(NCORES, NBLK, 2)
    tiles = -(-counts // 128)                              # ceil
    tlo = tuple(int(t) for t in tiles[:, :, 0].max(axis=0))
    thi = tuple(int(t) for t in tiles[:, :, 1].max(axis=0))
    CB = sum(tlo) + sum(thi)

    # per-core slot tables in global column-block (cb) order
    feat_idx = np.zeros((NCORES, CB * 128), np.int16)
    dloc = np.full((NCORES, 128, CB), 128.0, np.float32)
    # transposed one-hot per tile: oht[c, d, cb, e] = 1 iff dl(slot cb*128+e)==d
    oht = np.zeros((NCORES, 128, CB, 128), NP_FP8)

    starts = np.zeros(NCORES * NBLK * 2 + 1, np.int64)
    starts[1:] = np.cumsum(counts.reshape(-1))

    # cb offset of each (block, half) segment, same for every core
    seg_off = {}
    cb0 = 0
    for g0 in range(0, NBLK, GBLK):
        blocks = list(range(g0, min(NBLK, g0 + GBLK)))
        off = cb0
        for b in blocks:
            seg_off[(b, 0)] = off
            off += tlo[b]
        for b in blocks:
            seg_off[(b, 1)] = off
            off += thi[b]
        cb0 = off
    assert cb0 == CB

    one8 = NP_FP8(1.0)
    for c in range(NCORES):
        for b in range(NBLK):
            for hf in range(2):
                gi = (c * NBLK + b) * 2 + hf
                s, e = starts[gi], starts[gi + 1]
                n = int(e - s)
                if n == 0:
                    continue
                off = seg_off[(b, hf)]
                k = np.arange(n) + off * 128
                fi = (trow[s:e] - hf * HALF).astype(np.int16)
                feat_idx[c, k] = fi
                p = k % 128
                t = k // 128
                dl = (dst[s:e] - (c * SHARD + b * 128)).astype(np.int64)
                dloc[c, p, t] = dl.astype(np.float32)
                oht[c, dl, t, p] = one8

    in_maps = []
    for c in range(NCORES):
        xs = np.zeros((PAD_SHARD, D), np.float32)
        xs[0:SHARD] = x[c * SHARD : (c + 1) * SHARD]
        in_maps.append(
            {
                "x_shard": xs,
                "wext": wext.astype(NP_BF16),
                "c2row": c2.astype(NP_BF16),
                "ones1": ones1.astype(NP_BF16),
                "ident": ident.astype(NP_BF16),
                "iota": iota.astype(NP_BF16),
                "iotap": iotap,
                "feat_idx": _wrap_idx(feat_idx[c]),
                "dloc": np.ascontiguousarray(dloc[c]),
                "oht": np.ascontiguousarray(oht[c].reshape(128, CB * 128)),
            }
        )
    return tlo, thi, in_maps


_PROGRAM_CACHE = {}


def kernel(x, edge_index, edge_attr, h, batch, ln_gamma, ln_beta, W, att_src,
           att_dst, bias):
    x = np.asarray(x, dtype=np.float32)
    edge_index = np.asarray(edge_index)
    h = np.asarray(h)
    ln_gamma = np.asarray(ln_gamma, dtype=np.float32)
    ln_beta = np.asarray(ln_beta, dtype=np.float32)
    W = np.asarray(W, dtype=np.float32)
    att_src = np.asarray(att_src, dtype=np.float32)
    att_dst = np.asarray(att_dst, dtype=np.float32)
    bias = np.asarray(bias, dtype=np.float32)

    tlo, thi, in_maps = _host_prep(
        x, edge_index, ln_gamma, ln_beta, W, att_src, att_dst, bias
    )
    key = (tlo, thi)
    if key not in _PROGRAM_CACHE:
        _PROGRAM_CACHE[key] = _build_program(tlo, thi)
    nc = _PROGRAM_CACHE[key]

    res = run_bass_kernel_spmd(nc, in_maps, core_ids=list(range(NCORES)))
    out = np.concatenate([res.results[c]["out_shard"] for c in range(NCORES)], axis=0)
    return out, h
